# revision 3
# baseline (speedup 1.0000x reference)
"""Distributed GQA attention kernel for one TRN2 chip (8 NeuronCores), v10c.

Problem: B=2, L=2048, HID=2048, H=32 q-heads, HKV=8 kv-heads, D=64,
rotary embedding, causal softmax, o-proj.

Sharding: core i -> batch b=i//4, TP rank r=i%4.  Each core computes
8 q-heads / 2 kv-heads of its batch, all-gathers the attention outputs
(feature-major, bf16) within its 4-core TP group, then computes its
512 output columns of the o-proj.  Host assembles the full output.

Schedule (v6):
 - proj(tt) and attention(qT=tt) rounds interleaved; exp starts ~25us in.
 - RoPE rotate-half via a PE permutation matmul (SW input), no SBUF-SBUF
   DMAs; swap matmuls delayed one projection stream so the PE never
   waits on the PSUM->SBUF copy feeding them.
 - causal mask as post-exp 0/1 DVE multiply (Pool TENSOR_TENSOR measured
   3x slower than DVE, and it delays collectives sharing the queue).
 - AllGather split per jj into an early piece (qT0-2 columns, issued
   right after round 2) and a late piece (qT3 columns, right after each
   attn(jj,3)); the serialized ~130us CC chain hides behind round-3
   attention + o-proj.  Bounce DMAs issue from the gpsimd queue.
 - o-proj in 4 quarters (one per gathered jj group), partial sums
   accumulated in SBUF fp32 via DVE adds.

All matmuls bf16 with fp32 PSUM accumulation.  Softmax skips the
row-max (logits bounded ~|6|) and gets row sums free via a 64-wide
ones block in V's stationary operand; normalization is a DVE
reciprocal + multiply.
"""

import sys

sys.path.insert(0, "/opt/trn_rl_repo")

import numpy as np
import ml_dtypes

B, L, HID = 2, 2048, 2048
H, HKV, D = 32, 8, 64
N_CORES = 8
TP = 4           # tensor-parallel group size
HL = 8           # q heads per core
CW = 512         # o-proj output columns per core
TT = 4           # t tiles of 512 over L
CCH = HID // 128 # contraction chunks (16)
LE = 3 * 512     # early-AG columns (qT 0-2)
BF16 = ml_dtypes.bfloat16

_cache = {}


def _build_graph(dbg=None):
    import concourse.bass as bass
    import concourse.tile as tile
    from concourse import bacc, mybir

    dt = mybir.dt
    f32, bf16 = dt.float32, dt.bfloat16

    nc = bacc.Bacc("TRN2", target_bir_lowering=False, debug=False,
                   num_devices=N_CORES)

    xT = nc.dram_tensor("xT", [HID, L], bf16, kind="ExternalInput")
    WqT = nc.dram_tensor("WqT", [HID, HL * D], bf16, kind="ExternalInput")
    WkT = nc.dram_tensor("WkT", [HID, 128], bf16, kind="ExternalInput")
    WvT = nc.dram_tensor("WvT", [HID, 128], bf16, kind="ExternalInput")
    WoT = nc.dram_tensor("WoT", [HID, CW], bf16, kind="ExternalInput")
    SW = nc.dram_tensor("SW", [128, 128], bf16, kind="ExternalInput")
    C1q = nc.dram_tensor("C1q", [128, L], bf16, kind="ExternalInput")
    C2q = nc.dram_tensor("C2q", [128, L], bf16, kind="ExternalInput")
    C1k = nc.dram_tensor("C1k", [128, L], bf16, kind="ExternalInput")
    C2k = nc.dram_tensor("C2k", [128, L], bf16, kind="ExternalInput")
    out = nc.dram_tensor("out", [CW, L], f32, kind="ExternalOutput")
    dbg_shapes = {"qq": [128, HL // 2 * L], "kk": [128, L],
                  "v2": [128, CCH * 256], "ao": [128, HL // 2 * L]}
    dbg_t = (nc.dram_tensor("dbg", dbg_shapes[dbg], bf16,
                            kind="ExternalOutput") if dbg else None)

    with tile.TileContext(nc) as tc:
        with (
            tc.tile_pool(name="persist", bufs=1) as persist,
            tc.tile_pool(name="ps_s", bufs=2, space="PSUM") as ps_s,
            tc.tile_pool(name="ps_o", bufs=2, space="PSUM") as ps_o,
            tc.tile_pool(name="pp", bufs=4) as pp,
            tc.tile_pool(name="aog", bufs=8) as aogp,
            tc.tile_pool(name="dram", bufs=1, space="DRAM") as dram,
        ):
            # ---- persistent SBUF tensors ----
            qq = persist.tile([128, HL // 2 * L], bf16)      # roped Q^T, 2MB
            kk = persist.tile([128, L], bf16)                # roped K^T (2 kv)
            v2t = persist.tile([128, L], bf16)               # V^T staging
            v2 = persist.tile([128, CCH * 256], bf16)        # [V|1|V|1] per kt
            ao = persist.tile([128, HL // 2 * L], bf16)      # attn out^T
            sw = persist.tile([128, 128], bf16)
            c1q = persist.tile([128, L], bf16)
            c2q = persist.tile([128, L], bf16)
            c1k = persist.tile([128, L], bf16)
            c2k = persist.tile([128, L], bf16)
            mask01 = persist.tile([128, 4 * 512], bf16)
            wo_sb = persist.tile([128, CCH * 512], bf16)

            # ones blocks of v2 (columns 64:128 and 192:256 of each kt group)
            for off in (64, 192):
                ones_view = bass.AP(v2.tensor, v2.offset + off,
                                    [v2.ap[0], [256, CCH], [1, 64]])
                nc.gpsimd.memset(ones_view, 1.0)

            # build mask01 from the -48/0 bmask pattern (predicate: masked
            # where q < 128*dj + k within the [128,512] diag block)
            with tc.tile_pool(name="mk", bufs=1) as mk:
                bm = mk.tile([128, 2048], bf16)
                nc.gpsimd.memset(bm[:], -48.0)
                for j in range(4):
                    nc.gpsimd.affine_select(
                        out=bm[:, j * 512:(j + 1) * 512],
                        in_=bm[:, j * 512:(j + 1) * 512],
                        pattern=[[-1, 512]], compare_op=mybir.AluOpType.is_gt,
                        fill=0.0, base=128 * j, channel_multiplier=1)
                for j in range(4):
                    nc.vector.tensor_scalar(
                        mask01[:, j * 512:(j + 1) * 512],
                        bm[:, j * 512:(j + 1) * 512],
                        1.0 / 48.0, 1.0,
                        mybir.AluOpType.mult, mybir.AluOpType.add)

            # ---------- attention building block ----------
            def attn_qT(jj, qT):
                nkt = 4 * qT + 4
                qs = slice(jj * L + qT * 512, jj * L + (qT + 1) * 512)
                o0 = ps_o.tile([128, 512], f32, tag="o", name=f"o0_{jj}_{qT}")
                o1 = ps_o.tile([128, 512], f32, tag="o", name=f"o1_{jj}_{qT}")

                def emit_av(kt, p, off):
                    nc.tensor.matmul(
                        o0[:, off:512], lhsT=v2[:, kt * 256:kt * 256 + 128],
                        rhs=p[:, off:512], start=(kt == 0),
                        stop=(kt == nkt - 1), skip_group_check=True)
                    nc.tensor.matmul(
                        o1[:, off:512],
                        lhsT=v2[:, kt * 256 + 128:(kt + 1) * 256],
                        rhs=p[:, 512 + off:1024], start=(kt == 0),
                        stop=(kt == nkt - 1), skip_group_check=True)

                prev = None
                for kt in range(nkt):
                    ksl = slice(kt * 128, (kt + 1) * 128)
                    dj = kt - 4 * qT
                    # diag blocks: only columns q >= dj*128 can be unmasked
                    off = max(dj, 0) * 128
                    wv_ = 512 - off
                    sb = ps_s.tile([128, 1024], f32, tag="s",
                                   name=f"sb_{jj}_{qT}_{kt}")
                    nc.tensor.matmul(sb[:, off:512], lhsT=kk[0:64, ksl],
                                     rhs=qq[0:64, qs.start + off:qs.stop],
                                     start=True, stop=True,
                                     tile_position=(0, 0))
                    nc.tensor.matmul(sb[:, 512 + off:1024],
                                     lhsT=kk[64:128, ksl],
                                     rhs=qq[64:128, qs.start + off:qs.stop],
                                     start=True, stop=True,
                                     tile_position=(64, 0))
                    p = pp.tile([128, 1024], bf16, tag="p",
                                name=f"p_{jj}_{qT}_{kt}")
                    pv = bass.AP(p.tensor, p.offset + off,
                                 [p.ap[0], [512, 2], [1, wv_]])
                    sv = bass.AP(sb.tensor, sb.offset + off,
                                 [sb.ap[0], [512, 2], [1, wv_]])
                    nc.scalar.activation(pv, sv,
                                         mybir.ActivationFunctionType.Exp)
                    if dj >= 0:
                        m3 = bass.AP(mask01.tensor,
                                     mask01.offset + dj * 512 + off,
                                     [mask01.ap[0], [0, 2], [1, wv_]])
                        nc.vector.tensor_tensor(pv, pv, m3,
                                                mybir.AluOpType.mult)
                    if prev is not None:
                        emit_av(*prev)
                    prev = (kt, p, off)
                emit_av(*prev)

                # normalize (approx-recip full tile; rows 64:128 hold the
                # replicated sums - base!=0 slices break the custom-DVE op)
                rc = pp.tile([128, 512], f32, tag="rc", bufs=2,
                             name=f"rc_{jj}_{qT}")
                nc.vector.reciprocal_approx_fast(rc[:], o0[:])
                nc.vector.tensor_tensor(ao[0:64, qs], o0[0:64, :],
                                        rc[64:128, :], mybir.AluOpType.mult)
                rc2 = pp.tile([128, 512], f32, tag="rc", bufs=2,
                              name=f"rc2_{jj}_{qT}")
                nc.vector.reciprocal_approx_fast(rc2[:], o1[:])
                nc.vector.tensor_tensor(ao[64:128, qs], o1[0:64, :],
                                        rc2[64:128, :], mybir.AluOpType.mult)

            # ---------- split collectives ----------
            bnc_e = [dram.tile([128, LE], bf16, name=f"bnc_e{j}")
                     for j in range(4)]
            gath_e = [dram.tile([TP * 128, LE], bf16, name=f"gath_e{j}")
                      for j in range(4)]
            bnc_l = [dram.tile([128, 512], bf16, name=f"bnc_l{j}")
                     for j in range(4)]
            gath_l = [dram.tile([TP * 128, 512], bf16, name=f"gath_l{j}")
                      for j in range(4)]

            aols = {}

            def ag(j, late):
                bnc = (bnc_l if late else bnc_e)[j]
                gt = (gath_l if late else gath_e)[j]
                c0 = j * L + (LE if late else 0)
                w = 512 if late else LE
                for g in range(2):
                    nc.gpsimd.dma_start(bnc[g * 64:(g + 1) * 64, :],
                                        ao[g * 64:(g + 1) * 64, c0:c0 + w])
                nc.gpsimd.collective_compute(
                    "AllGather", mybir.AluOpType.bypass,
                    replica_groups=[[0, 1, 2, 3], [4, 5, 6, 7]],
                    ins=[bnc.opt()], outs=[gt.opt()])
                if late:
                    # prefetch the gathered late columns right behind the CC
                    for c in range(TP):
                        aol = aogp.tile([128, 512], bf16, tag="aol",
                                        bufs=16, name=f"aol_{j}_{c}")
                        nc.gpsimd.dma_start(
                            aol[:], gath_l[j][c * 128:(c + 1) * 128, :])
                        aols.setdefault(j, []).append(aol)

            # =========== interleaved projections + attention rounds =======
            with (
                tc.tile_pool(name="wsb", bufs=1) as wsb,
                tc.tile_pool(name="xtp", bufs=2) as xtp,
                tc.tile_pool(name="rope", bufs=2) as rope,
                tc.tile_pool(name="ps_mm", bufs=2, space="PSUM") as ps_mm,
            ):
                wk_sb = wsb.tile([128, CCH * 128], bf16)
                nc.scalar.dma_start(
                    wk_sb[:].rearrange("p (c m) -> p c m", m=128),
                    WkT[:].rearrange("(c p) m -> p c m", p=128))
                wv_sb = wsb.tile([128, CCH * 128], bf16)
                nc.scalar.dma_start(
                    wv_sb[:].rearrange("p (c m) -> p c m", m=128),
                    WvT[:].rearrange("(c p) m -> p c m", p=128))
                wq_sb = wsb.tile([128, CCH * 512], bf16)
                wq_v = WqT[:].rearrange("(c p) m -> p c m", p=128)
                wq_s = wq_sb[:].rearrange("p (c m) -> p c m", m=512)
                for cb in range(4):
                    nc.scalar.dma_start(wq_s[:, 4 * cb:4 * (cb + 1)],
                                        wq_v[:, 4 * cb:4 * (cb + 1)])
                nc.scalar.dma_start(
                    wo_sb[:].rearrange("p (c m) -> p c m", m=512),
                    WoT[:].rearrange("(c p) m -> p c m", p=128))
                for t_sb, t_dr in ((sw, SW), (c1q, C1q), (c2q, C2q),
                                   (c1k, C1k), (c2k, C2k)):
                    nc.scalar.dma_start(t_sb[:], t_dr[:])

                xT_view = xT[:].rearrange("(c p) t -> p c t", p=128)

                def proj_tt(tt, per_m_hook=None):
                    """K, V, then Q m-tiles.  RoPE rotate-half via PE
                    permutation matmul, delayed one stream (pend FIFO).
                    per_m_hook(m) is called right after Q m's rope flush
                    (round 3 uses it to interleave attention + late AGs)."""
                    ts = slice(tt * 512, (tt + 1) * 512)
                    xt = xtp.tile([128, CCH * 512], bf16, tag="xt")
                    xt_s = xt[:].rearrange("p (c t) -> p c t", t=512)
                    for cb in range(4):
                        nc.sync.dma_start(xt_s[:, 4 * cb:4 * (cb + 1)],
                                          xT_view[:, 4 * cb:4 * (cb + 1), ts])

                    pend = []

                    def flush_swap():
                        if not pend:
                            return
                        raw, c1s, c2s, dst, nm = pend.pop(0)
                        ps2 = ps_mm.tile([128, 512], f32, tag="mm",
                                         name=f"ps2_{tt}_{nm}")
                        nc.tensor.matmul(ps2[:], lhsT=sw[:], rhs=raw[:],
                                         start=True, stop=True)
                        tmp = rope.tile([128, 512], bf16, tag="rtmp", bufs=3,
                                        name=f"rtmp{tt}_{nm}")
                        nc.vector.tensor_tensor(tmp[:], raw[:], c1s,
                                                mybir.AluOpType.mult)
                        rsw = rope.tile([128, 512], bf16, tag="rsw", bufs=3,
                                        name=f"rsw{tt}_{nm}")
                        nc.vector.tensor_tensor(rsw[:], ps2[:], c2s,
                                                mybir.AluOpType.mult)
                        nc.vector.tensor_tensor(dst, tmp[:], rsw[:],
                                                mybir.AluOpType.add)

                    # --- K ---
                    kraw = rope.tile([128, 512], bf16, tag="qraw", bufs=3,
                                     name=f"kraw{tt}")
                    psk = ps_mm.tile([128, 512], f32, tag="mm",
                                     name=f"psk{tt}")
                    for c in range(CCH):
                        nc.tensor.matmul(
                            psk[:], lhsT=wk_sb[:, c * 128:(c + 1) * 128],
                            rhs=xt[:, c * 512:(c + 1) * 512],
                            start=(c == 0), stop=(c == CCH - 1))
                    nc.vector.tensor_scalar_mul(kraw[:], psk[:], 1.0)
                    pend.append((kraw, c1k[:, ts], c2k[:, ts], kk[:, ts],
                                 "k"))

                    # --- V ---
                    psv = ps_mm.tile([128, 512], f32, tag="mm",
                                     name=f"psv{tt}")
                    for c in range(CCH):
                        nc.tensor.matmul(
                            psv[:], lhsT=wv_sb[:, c * 128:(c + 1) * 128],
                            rhs=xt[:, c * 512:(c + 1) * 512],
                            start=(c == 0), stop=(c == CCH - 1))
                    flush_swap()          # K rope (kk ready before attention)
                    nc.vector.tensor_scalar_mul(v2t[:, ts], psv[:], 1.0)
                    for g in range(2):
                        v2_dst = bass.AP(v2.tensor,
                                         v2.offset + (4 * tt) * 256 + g * 128,
                                         [v2.ap[0], [256, 4], [1, 64]])
                        nc.sync.dma_start_transpose(
                            v2_dst, v2t[g * 64:(g + 1) * 64, ts])

                    # --- Q m-tiles ---
                    for m in range(4):
                        psq = ps_mm.tile([128, 512], f32, tag="mm",
                                         name=f"psq{tt}_{m}")
                        for c in range(CCH):
                            nc.tensor.matmul(
                                psq[:],
                                lhsT=wq_sb[:, c * 512 + m * 128:
                                           c * 512 + (m + 1) * 128],
                                rhs=xt[:, c * 512:(c + 1) * 512],
                                start=(c == 0), stop=(c == CCH - 1))
                        flush_swap()      # Q m-1 (or V-round K leftover)
                        qraw = rope.tile([128, 512], bf16, tag="qraw",
                                         bufs=3, name=f"qraw{tt}_{m}")
                        nc.vector.tensor_scalar_mul(qraw[:], psq[:], 1.0)
                        pend.append((qraw, c1q[:, ts], c2q[:, ts],
                                     qq[:, m * L + tt * 512:
                                        m * L + (tt + 1) * 512], f"q{m}"))
                        if per_m_hook is not None and m > 0:
                            per_m_hook(m - 1)
                    flush_swap()
                    if per_m_hook is not None:
                        per_m_hook(3)

                for r in range(TT - 1):
                    proj_tt(r)
                    for jj in range(4):
                        attn_qT(jj, r)
                # early AGs: jj columns of qT0-2 are final after round 2
                for j in range(4):
                    ag(j, late=False)

                def round3_hook(m):
                    # m's rope flushed -> qq(m, tt3) ready; kk/v2 flushed
                    # before the Q loop
                    attn_qT(m, 3)
                    ag(m, late=True)

                proj_tt(3, per_m_hook=round3_hook)

            # ====== o-proj in 4 quarters with SBUF fp32 accumulation ======
            with (
                tc.tile_pool(name="sta", bufs=16) as stap,
                tc.tile_pool(name="ost", bufs=2) as ostp,
                tc.tile_pool(name="ps_op", bufs=2, space="PSUM") as ps_op,
            ):
                aoes = {}

                def load_aoe(j):
                    for c in range(TP):
                        aoe = aogp.tile([128, LE], bf16, tag="aoe",
                                        bufs=8, name=f"aoe_{j}_{c}")
                        nc.sync.dma_start(
                            aoe[:], gath_e[j][c * 128:(c + 1) * 128, :])
                        aoes.setdefault(j, []).append(aoe)

                load_aoe(0)
                acc = {}
                for j in range(4):
                    if j + 1 < 4:
                        load_aoe(j + 1)
                    for tt2 in range(TT):
                        tsl = slice(tt2 * 512, (tt2 + 1) * 512)
                        for ct in range(4):
                            pso = ps_op.tile([128, 512], f32, tag="op",
                                             name=f"pso_{j}_{tt2}_{ct}")
                            for c in range(TP):
                                rhs = (aoes[j][c][:, tsl] if tt2 < 3
                                       else aols[j][c][:])
                                nc.tensor.matmul(
                                    pso[:],
                                    lhsT=wo_sb[:, (4 * j + c) * 512
                                               + ct * 128:
                                               (4 * j + c) * 512
                                               + (ct + 1) * 128],
                                    rhs=rhs, start=(c == 0),
                                    stop=(c == TP - 1))
                            if j == 0:
                                a = stap.tile([128, 512], f32, tag="acc",
                                              name=f"acc{tt2}_{ct}")
                                nc.vector.tensor_scalar_mul(a[:], pso[:], 1.0)
                                acc[(tt2, ct)] = a
                            elif j < 3:
                                nc.vector.tensor_tensor(
                                    acc[(tt2, ct)][:], pso[:],
                                    acc[(tt2, ct)][:], mybir.AluOpType.add)
                            else:
                                ost = ostp.tile([128, 512], f32, tag="ost",
                                                name=f"ost{tt2}_{ct}")
                                nc.vector.tensor_tensor(
                                    ost[:], pso[:], acc[(tt2, ct)][:],
                                    mybir.AluOpType.add)
                                nc.sync.dma_start(
                                    out[ct * 128:(ct + 1) * 128, tsl], ost[:])

            if dbg:
                src = {"qq": qq, "kk": kk, "v2": v2, "ao": ao}[dbg]
                nc.sync.dma_start(dbg_t[:], src[:])

    nc.compile()
    return nc


def _host_prep(hidden_states, cos, sin, Wq, Wk, Wv, Wo):
    """Build the 8 per-core input maps (all host-side slicing/transposes)."""
    scale = float(D) ** -0.5
    # rope coefficient tables [128, L]: 4 groups of 32 rows (d 0:32 pattern)
    cosT = cos[:, :32].T.astype(np.float32)          # [32, L]
    sinT = sin[:, :32].T.astype(np.float32)
    c1 = np.tile(cosT, (4, 1))                       # [128, L]
    c2 = np.concatenate([-sinT, sinT, -sinT, sinT], axis=0)
    swm = np.zeros((128, 128), np.float32)
    for a, b in ((0, 32), (32, 0), (64, 96), (96, 64)):
        swm[np.arange(a, a + 32), np.arange(b, b + 32)] = 1.0
    tables = {
        "SW": swm.astype(BF16),
        "C1q": (c1 * scale).astype(BF16), "C2q": (c2 * scale).astype(BF16),
        "C1k": c1.astype(BF16), "C2k": c2.astype(BF16),
    }
    xTb = [np.ascontiguousarray(hidden_states[b].T).astype(BF16)
           for b in range(B)]
    in_maps = []
    for i in range(N_CORES):
        b, r = divmod(i, TP)
        # Wq rows reordered: M-tile m = heads (8r+m, 8r+4+m)
        rows = []
        for m in range(4):
            rows.append(Wq[(8 * r + m) * D:(8 * r + m + 1) * D])
            rows.append(Wq[(8 * r + 4 + m) * D:(8 * r + 4 + m + 1) * D])
        WqT_i = np.ascontiguousarray(np.concatenate(rows, 0).T).astype(BF16)
        WkT_i = np.ascontiguousarray(
            Wk[2 * r * D:(2 * r + 2) * D].T).astype(BF16)
        WvT_i = np.ascontiguousarray(
            Wv[2 * r * D:(2 * r + 2) * D].T).astype(BF16)
        # o-proj k-rows quarter-major: quarter j rows j*512..: rank=R//128,
        # g=(R%128)//64, d=R%64 -> feature = head(8*rank + j + 4*g)*64 + d
        RR = np.arange(2048)
        j_, Rq = RR // 512, RR % 512
        perm = ((8 * (Rq // 128) + j_ + 4 * ((Rq % 128) // 64)) * D
                + Rq % 64)
        WoT_i = np.ascontiguousarray(
            Wo[CW * r:CW * (r + 1), :].T[perm]).astype(BF16)
        in_maps.append({
            "xT": xTb[b], "WqT": WqT_i, "WkT": WkT_i, "WvT": WvT_i,
            "WoT": WoT_i, **tables,
        })
    return in_maps


def kernel(hidden_states, cos, sin, Wq, Wk, Wv, Wo, _want_profile=False):
    from concourse.bass_utils import run_bass_kernel_spmd

    if "nc" not in _cache:
        _cache["nc"] = _build_graph()
    nc = _cache["nc"]
    in_maps = _host_prep(np.asarray(hidden_states), np.asarray(cos),
                         np.asarray(sin), np.asarray(Wq), np.asarray(Wk),
                         np.asarray(Wv), np.asarray(Wo))
    res = run_bass_kernel_spmd(nc, in_maps, list(range(N_CORES)),
                               trace=_want_profile)
    # assemble: core (b, r) holds out^T [512, L] = cols [512r, 512r+512) of b
    full = np.empty((B, L, HID), np.float32)
    for i in range(N_CORES):
        b, r = divmod(i, TP)
        full[b, :, CW * r:CW * (r + 1)] = res.results[i]["out"].T
    if _want_profile:
        return full, res
    return full
